# revision 27
# baseline (speedup 1.0000x reference)
"""CrossAttentionFusion kernel for Trainium2 (8 NeuronCores, Bass/Tile).

Computation (matches the reference nn.Module):
  image_proj = relu(BN(1x1conv(image_features, image_w)))   # (B,128,H,W)
  lidar_proj = relu(BN(1x1conv(lidar_features, lidar_w)))   # (B,128,H,W)
  per (batch, 2048-pixel chunk): q = image_proj, k = v = lidar_proj
  attn_out = softmax(q k^T / sqrt(128)) @ k
  out = w0 * image_proj + w1 * attn_out,  w = softmax(modality_weights)

Sharding: the 16 independent (batch, chunk) attention problems are
distributed 2-per-core across 8 cores; each core also computes the
projections for its own pixels.  Host gathers the 8 outputs.

Per-core kernel layout notes (bf16 pipeline):
  - All matmul operands are bf16; PSUM accumulation stays fp32.  Host
    converts inputs to bf16 and folds the BN scale into the weights, so
    the projection epilogue is a single DVE tensor_scalar:
    max(psum + bias, 0) -> bf16.
  - Matmuls are 1024 wide (output spans two PSUM banks), halving the
    instruction + LDWEIGHTS count vs 512-wide halves.
  - Scores are computed k-major: ps[kslice, q]; exp() on the scalar
    engine writes bf16 (the scalar engine is the pace-setter: ~1.3us
    per 1024-wide exp, 64 of them).
  - AV uses transposed-K tiles: po[c, q] += Kpix_i^T @ ET_i, lagging
    LOOKAHEAD slices behind the score stream; the slice loop runs
    globally across both q-blocks so the PE never drains at block
    boundaries.
  - softmax denominator: binary-tree bf16 adds of the 16 ET tiles on
    the vector engine (S), then (ones/w1)^T @ S broadcast-sums across
    partitions on the PE; linv = 1/pl then already carries w1.
  - w0 is folded into the image BN affine (relu(w0*x) = w0*relu(x));
    the exp scale compensates with 1/(w0*sqrt(C)).
  - Output written per-qb as bf16; host casts back to fp32.
"""

import math
import os
import sys
from contextlib import ExitStack

import ml_dtypes
import numpy as np

sys.path.insert(0, "/opt/trn_rl_repo")

import concourse.bass as bass  # noqa: E402
import concourse.tile as tile  # noqa: E402
from concourse import bacc, bass_isa, mybir  # noqa: E402
from concourse.bass import ds, ts  # noqa: E402
from concourse.bass_utils import run_bass_kernel_spmd  # noqa: E402

F32 = mybir.dt.float32
BF16 = mybir.dt.bfloat16
NPBF16 = ml_dtypes.bfloat16

B, CL, CI, CO = 2, 256, 512, 128
H = W = 128
P = H * W                    # 16384 pixels per batch
CHUNK = 2048                 # attention chunk (pixels)
NCH = P // CHUNK             # 8 chunks per batch
NCORES = 8
UPC = (B * NCH) // NCORES    # units (b,chunk) per core = 2
EPS = 1e-5
QB = 512                     # q-block width (one 1-bank PSUM matmul)
NQB = CHUNK // QB            # 4
KSL = CHUNK // 128           # 16 k-pixel slices per chunk
NSL = NQB * KSL              # 32 (qb, slice) score tiles per unit
NCI_IMG = CI // 128          # 4 contraction slices for image proj
NCI_LID = CL // 128          # 2 for lidar proj

_PROGRAM = None              # compiled Bass program, built once per process
LAST_RESULTS = None          # BassKernelResults of the last kernel() call


def _build_program():
    nc = bacc.Bacc("TRN2", target_bir_lowering=False, debug=False,
                   num_devices=NCORES)

    # Per-core DRAM inputs (pre-sharded, bf16 on host, constants packed so
    # the startup path is few DMA issues).
    ximg = nc.dram_tensor("ximg", [UPC, NCI_IMG, 128, CHUNK], BF16,
                          kind="ExternalInput").ap()
    xlid = nc.dram_tensor("xlid", [UPC, NCI_LID, 128, CHUNK], BF16,
                          kind="ExternalInput").ap()
    wimg = nc.dram_tensor("wimg", [128, NCI_IMG * CO], BF16,
                          kind="ExternalInput").ap()
    wlid = nc.dram_tensor("wlid", [128, NCI_LID * CO], BF16,
                          kind="ExternalInput").ap()
    # columns: img_bias, lid_bias, exp_scale, w1
    scal = nc.dram_tensor("scal", [128, 4], F32, kind="ExternalInput").ap()
    # columns 0:128 identity, 128:256 ones/w1
    idon = nc.dram_tensor("idon", [128, 256], BF16, kind="ExternalInput").ap()
    y = nc.dram_tensor("y", [UPC, NQB, CO, QB], BF16, kind="ExternalOutput").ap()

    with tile.TileContext(nc) as tc, ExitStack() as ctx:
        const = ctx.enter_context(tc.tile_pool(name="const", bufs=1))
        xi_pool = ctx.enter_context(tc.tile_pool(name="xi", bufs=UPC * NCI_IMG))
        xl_pool = ctx.enter_context(tc.tile_pool(name="xl", bufs=UPC * NCI_LID))
        proj_pool = ctx.enter_context(tc.tile_pool(name="proj", bufs=4))
        kp_pool = ctx.enter_context(tc.tile_pool(name="kp", bufs=4))
        et_pool = ctx.enter_context(tc.tile_pool(name="et", bufs=6))
        tree_pool = ctx.enter_context(tc.tile_pool(name="tree", bufs=10))
        misc_pool = ctx.enter_context(tc.tile_pool(name="misc", bufs=3))
        res_pool = ctx.enter_context(tc.tile_pool(name="res", bufs=3))
        # PSUM (8 banks of [128, 2KB]):
        #   mm 2x[128,1536]f32 (6 banks) - score groups of 3 k-slices,
        #     double buffered, so exp() runs 1536 wide; also borrowed for
        #     the projection groups ([128,512] each)
        #   av 2x[128,512]f32 (2 banks) - AV accumulation + the lb tile,
        #     plus the transpose/img-tail projection scratch ([128,1024]bf16
        #     / [128,512]f32 both fit a bank)
        mm_psum = ctx.enter_context(tc.tile_pool(name="mmps", bufs=2, space="PSUM"))
        av_psum = ctx.enter_context(tc.tile_pool(name="avps", bufs=2, space="PSUM"))

        # ---- startup-critical DMAs on the sync queue, in dependency order
        # for the first projection group (lidar first); the rest spread over
        # the scalar and gpsimd queues, which are otherwise idle here ----
        xl = {0: [xl_pool.tile([128, CHUNK], BF16, name=f"xl_0_{ci}", tag="xl")
                  for ci in range(NCI_LID)]}
        xi = {0: [xi_pool.tile([128, CHUNK], BF16, name=f"xi_0_{ci}", tag="xi")
                  for ci in range(NCI_IMG)]}
        for ci in range(NCI_LID):
            nc.sync.dma_start(xl[0][ci][:, ds(0, 1024)], xlid[0, ci, :, ds(0, 1024)])
        wlid_t = const.tile([128, NCI_LID * CO], BF16)
        nc.sync.dma_start(wlid_t[:], wlid)
        scal_t = const.tile([128, 4], F32)
        nc.sync.dma_start(scal_t[:], scal)
        img_b = scal_t[:, ds(0, 1)]
        lid_b = scal_t[:, ds(1, 1)]
        esc = scal_t[:, ds(2, 1)]
        for ci in range(NCI_LID):
            nc.scalar.dma_start(xl[0][ci][:, ds(1024, 1024)],
                                xlid[0, ci, :, ds(1024, 1024)])
        wimg_t = const.tile([128, NCI_IMG * CO], BF16)
        nc.gpsimd.dma_start(wimg_t[:], wimg)
        idon_t = const.tile([128, 256], BF16)
        nc.gpsimd.dma_start(idon_t[:], idon)
        ident_t = idon_t[:, ds(0, 128)]
        ones_t = idon_t[:, ds(128, 128)]
        # pre-trigger the activation table load while DMAs are in flight
        warm = const.tile([128, 4], F32)
        nc.scalar.activation(warm[:], scal_t[:],
                             mybir.ActivationFunctionType.Relu)
        for ci in range(NCI_IMG):
            nc.gpsimd.dma_start(xi[0][ci][:, ds(0, 1024)],
                                ximg[0, ci, :, ds(0, 1024)])
        for ci in range(NCI_IMG):
            nc.gpsimd.dma_start(xi[0][ci][:, ds(1024, 1024)],
                                ximg[0, ci, :, ds(1024, 1024)])
        for u in range(1, UPC):
            xl[u] = [xl_pool.tile([128, CHUNK], BF16, name=f"xl_{u}_{ci}",
                                  tag="xl") for ci in range(NCI_LID)]
            xi[u] = [xi_pool.tile([128, CHUNK], BF16, name=f"xi_{u}_{ci}",
                                  tag="xi") for ci in range(NCI_IMG)]
            for ci in range(NCI_LID):
                nc.gpsimd.dma_start(xl[u][ci][:], xlid[u, ci])
            for ci in range(NCI_IMG):
                nc.gpsimd.dma_start(xi[u][ci][:], ximg[u, ci])

        qT = {}
        kT = {}
        kpw = {}

        def proj_group(u, dst, w_t, nci, xsrc, q0, bias, on_act):
            """One [128, QB] projection group: matmul accumulate + relu
            (PSUM always from the score pool; insertions are emitted in
            parity-preserving pairs).  The epilogue runs on ACT where it
            fills idle time, or on the DVE when slotted into the exp
            stream."""
            ps = pool_tile = mm_psum.tile(
                [128, QB], F32, name=f"pj_{u}_{dst.tensor.name}_{q0}",
                tag="ps")
            for ci in range(nci):
                nc.tensor.matmul(ps[:], w_t[:, ts(ci, CO)],
                                 xsrc[ci][:, ds(q0, QB)],
                                 start=(ci == 0), stop=(ci == nci - 1))
            if on_act:
                nc.scalar.activation(dst[:, ds(q0, QB)], ps[:],
                                     mybir.ActivationFunctionType.Relu,
                                     bias=bias)
            else:
                nc.vector.tensor_scalar(dst[:, ds(q0, QB)], ps[:], bias, 0.0,
                                        op0=mybir.AluOpType.add,
                                        op1=mybir.AluOpType.max)

        def emit_proj_part1(u):
            """Lidar projection (all of kT) and the first half of qT, relu
            on the scalar engine (idle in this phase)."""
            qT[u] = proj_pool.tile([128, CHUNK], BF16, name=f"qT_{u}", tag="qT")
            kT[u] = proj_pool.tile([128, CHUNK], BF16, name=f"kT_{u}", tag="kT")
            kpw[u] = []
            for q0 in range(0, CHUNK, QB):
                proj_group(u, kT[u], wlid_t, NCI_LID, xl[u], q0, lid_b, True)
            for q0 in range(0, CHUNK // 2, QB):
                proj_group(u, qT[u], wimg_t, NCI_IMG, xi[u], q0, img_b, True)
            emit_dma_transposes(u)

        def emit_dma_transposes(u):
            """K pixel-major tiles via xbar DMA transposes on the (idle)
            DMA engines, issued from the sync queue -- zero PE cost."""
            for g in range(KSL // 8):
                kpt = kp_pool.tile([128, 8 * 128], BF16, name=f"kp_{u}_{g}",
                                   tag="kp")
                for k in range(8):
                    nc.sync.dma_start_transpose(kpt[:, ts(k, 128)],
                                                kT[u][:, ts(g * 8 + k, 128)])
                kpw[u].append(kpt)

        last_w = [None]

        def mm_dedup(out, lhsT, rhs, start, stop):
            key = (lhsT.tensor.name, lhsT.offset)
            inst = nc.tensor.matmul(out, lhsT, rhs, start=start, stop=stop)
            if last_w[0] == key:
                inst.ins.ldweights = True
            last_w[0] = key
            return inst

        # group schedule per unit: per q-block (QB wide), the 16 k-slices
        # are exp'd in groups of 3 (5x1536-wide + 1x512-wide)
        GRPS = []
        for qb in range(NQB):
            for s0 in range(0, KSL, 3):
                GRPS.append((qb, list(range(s0, min(s0 + 3, KSL)))))
        NG = len(GRPS)                     # 24 groups per unit
        GLA = 2                            # AV lags 2 exp groups

        emit_proj_part1(0)
        for u in range(UPC):
            ets = {}                       # (qb, slice) -> et view
            pos = {}
            lvl1 = {}
            lvl2 = {}
            lvl3 = {}
            done_tree = set()

            def tree_step(qb):
                """Emit any newly-possible S-tree adds for this q-block.
                Partial sums kept 4-way (t3[0-7], t2[8-11], t1[12-13],
                t1[14-15]) so only one DVE add separates the last exp from
                the lb matmul."""
                for p in range(KSL // 2):
                    i0, i1 = 2 * p, 2 * p + 1
                    if (qb, p) in done_tree or (qb, i0) not in ets \
                            or (qb, i1) not in ets:
                        continue
                    done_tree.add((qb, p))
                    t = tree_pool.tile([128, QB], BF16,
                                       name=f"t1_{u}_{qb}_{p}", tag="t1")
                    nc.vector.tensor_add(t[:], ets[(qb, i0)][:],
                                         ets[(qb, i1)][:])
                    lvl1[(qb, p)] = t
                    if p % 2 == 1 and p < 6:
                        t2 = tree_pool.tile([128, QB], BF16,
                                            name=f"t2_{u}_{qb}_{p}", tag="t1")
                        nc.vector.tensor_add(t2[:], lvl1[(qb, p - 1)][:], t[:])
                        lvl2[(qb, p // 2)] = t2
                        if p == 3:
                            t3 = tree_pool.tile([128, QB], BF16,
                                                name=f"t3_{u}_{qb}", tag="t1")
                            nc.vector.tensor_add(t3[:], lvl2[(qb, 0)][:],
                                                 t2[:])
                            lvl3[qb] = t3

            def qb_epilogue(qb):
                """Denominator broadcast ((ones/w1)^T @ partial S tiles) into
                an AV-pool tile, then reciprocal + blend + store."""
                pl = av_psum.tile([128, QB], F32, name=f"pl_{u}_{qb}",
                                  tag="av")
                parts = [lvl3[qb], lvl2[(qb, 2)], lvl1[(qb, 6)],
                         lvl1[(qb, 7)]]
                for pi, part in enumerate(parts):
                    mm_dedup(pl[:], ones_t, part[:],
                             start=(pi == 0), stop=(pi == len(parts) - 1))
                linv = misc_pool.tile([128, QB], F32, name=f"linv_{u}_{qb}",
                                      tag="linv")
                tmp = misc_pool.tile([128, QB], F32, name=f"tmp_{u}_{qb}",
                                     tag="tmp")
                res = res_pool.tile([128, QB], BF16, name=f"res_{u}_{qb}",
                                    tag="res")
                nc.vector.reciprocal_approx_fast(linv[:], pl[:])
                nc.vector.tensor_mul(tmp[:], pos[qb][:], linv[:])
                nc.vector.tensor_add(res[:], tmp[:], qT[u][:, ts(qb, QB)])
                nc.sync.dma_start(y[u, qb], res[:])

            for G in range(NG + GLA):
                if G < NG:
                    qb, slices = GRPS[G]
                    w = len(slices) * QB
                    ps = mm_psum.tile([128, 3 * QB], F32,
                                      name=f"sc_{u}_{G}", tag="ps")
                    for si, s in enumerate(slices):
                        mm_dedup(ps[:, ts(si, QB)], kT[u][:, ts(s, 128)],
                                 qT[u][:, ts(qb, QB)], start=True, stop=True)
                    et = et_pool.tile([128, 3 * QB], BF16,
                                      name=f"et_{u}_{G}", tag="et")
                    nc.scalar.activation(et[:, ds(0, w)], ps[:, ds(0, w)],
                                         mybir.ActivationFunctionType.Exp,
                                         scale=esc)
                    for si, s in enumerate(slices):
                        ets[(qb, s)] = et[:, ts(si, QB)]
                    tree_step(qb)
                    # early-stream insertions: transposes + tail of the
                    # image projection (relu on DVE, PSUM from the AV pool)
                    if G == 3:
                        # both image-tail groups as one parity-preserving
                        # pair of score-pool allocations
                        proj_group(u, qT[u], wimg_t, NCI_IMG, xi[u],
                                   CHUNK // 2, img_b, False)
                        proj_group(u, qT[u], wimg_t, NCI_IMG, xi[u],
                                   CHUNK // 2 + QB, img_b, False)
                else:
                    if G == NG and u + 1 < UPC:
                        emit_proj_part1(u + 1)
                J = G - GLA
                if 0 <= J < NG:
                    qbj, jslices = GRPS[J]
                    if J % 6 == 0:
                        pos[qbj] = av_psum.tile([128, QB], F32,
                                                name=f"po_{u}_{qbj}",
                                                tag="av")
                    for s in jslices:
                        kslice = kpw[u][s // 8][:, ts(s % 8, 128)]
                        mm_dedup(pos[qbj][:], kslice, ets[(qbj, s)][:],
                                 start=(s == 0), stop=(s == KSL - 1))
                    if jslices[-1] == KSL - 1:
                        qb_epilogue(qbj)

    nc.compile()
    return nc


def _shard_inputs(inputs):
    """Build the 8 per-core input maps from the full input dict."""
    mw = np.asarray(inputs["modality_weights"], np.float64)
    e = np.exp(mw - mw.max())
    w = (e / e.sum()).astype(np.float64)
    w0, w1 = float(w[0]), float(w[1])

    def bn_fold(gamma, beta, mean, var, mul):
        g = np.asarray(gamma, np.float64)
        b = np.asarray(beta, np.float64)
        m = np.asarray(mean, np.float64)
        v = np.asarray(var, np.float64)
        scale = g / np.sqrt(v + EPS) * mul
        bias = (b - m * g / np.sqrt(v + EPS)) * mul
        return scale, bias.astype(np.float32)

    i_s, i_b = bn_fold(inputs["image_gamma"], inputs["image_beta"],
                       inputs["image_mean"], inputs["image_var"], w0)
    l_s, l_b = bn_fold(inputs["lidar_gamma"], inputs["lidar_beta"],
                       inputs["lidar_mean"], inputs["lidar_var"], 1.0)

    # weight slices with the BN scale folded in, packed [cin(128), ci*CO]
    wi = (np.asarray(inputs["image_w"], np.float64) * i_s[:, None]).T.reshape(
        NCI_IMG, 128, CO).transpose(1, 0, 2).reshape(128, NCI_IMG * CO)
    wi = np.ascontiguousarray(wi).astype(NPBF16)
    wl = (np.asarray(inputs["lidar_w"], np.float64) * l_s[:, None]).T.reshape(
        NCI_LID, 128, CO).transpose(1, 0, 2).reshape(128, NCI_LID * CO)
    wl = np.ascontiguousarray(wl).astype(NPBF16)

    scal = np.zeros((128, 4), np.float32)
    scal[:, 0] = i_b
    scal[:, 1] = l_b
    scal[:, 2] = 1.0 / (w0 * math.sqrt(CO))
    scal[:, 3] = w1
    idon = np.zeros((128, 256), np.float32)
    idon[:, :128] = np.eye(128, dtype=np.float32)
    # ones carry 1/w1 so linv = 1/pl = w1/denominator
    idon[:, 128:] = 1.0 / w1
    idon = idon.astype(NPBF16)

    # full features reshaped to (B, nchunks, C, 2048), bf16
    img = np.asarray(inputs["image_features"], np.float32).reshape(
        B, CI, NCH, CHUNK).astype(NPBF16)
    lid = np.asarray(inputs["lidar_features"], np.float32).reshape(
        B, CL, NCH, CHUNK).astype(NPBF16)

    in_maps = []
    for core in range(NCORES):
        ximg = np.empty((UPC, NCI_IMG, 128, CHUNK), NPBF16)
        xlid = np.empty((UPC, NCI_LID, 128, CHUNK), NPBF16)
        for ul in range(UPC):
            un = core * UPC + ul
            b, c = un // NCH, un % NCH
            ximg[ul] = img[b, :, c, :].reshape(NCI_IMG, 128, CHUNK)
            xlid[ul] = lid[b, :, c, :].reshape(NCI_LID, 128, CHUNK)
        in_maps.append({
            "ximg": ximg, "xlid": xlid, "wimg": wi, "wlid": wl,
            "scal": scal, "idon": idon,
        })
    return in_maps


def kernel(**inputs) -> np.ndarray:
    global _PROGRAM, LAST_RESULTS
    if _PROGRAM is None:
        _PROGRAM = _build_program()
    nc = _PROGRAM

    in_maps = _shard_inputs(inputs)
    trace = os.environ.get("BASS_KERNEL_TRACE", "0") == "1"
    tmpdir = os.environ.get("BASS_KERNEL_TRACE_DIR") or None
    if tmpdir:
        os.makedirs(tmpdir, exist_ok=True)
    results = run_bass_kernel_spmd(nc, in_maps, core_ids=list(range(NCORES)),
                                   trace=trace, tmpdir=tmpdir)
    LAST_RESULTS = results

    out = np.empty((B, CO, H, W), np.float32)
    outv = out.reshape(B, CO, NCH, NQB, QB)
    for core in range(NCORES):
        yc = np.asarray(results.results[core]["y"], dtype=np.float32)
        for ul in range(UPC):
            un = core * UPC + ul
            b, c = un // NCH, un % NCH
            outv[b, :, c, :, :] = yc[ul].transpose(1, 0, 2)
    return out


if __name__ == "__main__":
    rng = np.random.default_rng(0)
    inputs = {
        "lidar_features": rng.standard_normal((B, CL, H, W), np.float32),
        "image_features": rng.standard_normal((B, CI, H, W), np.float32),
        "lidar_w": rng.standard_normal((CO, CL), np.float32) * np.sqrt(2.0 / CO),
        "lidar_gamma": np.ones(CO, np.float32),
        "lidar_beta": np.zeros(CO, np.float32),
        "lidar_mean": rng.standard_normal(CO).astype(np.float32) * 0.1,
        "lidar_var": rng.uniform(0.5, 1.5, CO).astype(np.float32),
        "image_w": rng.standard_normal((CO, CI), np.float32) * np.sqrt(2.0 / CO),
        "image_gamma": np.ones(CO, np.float32),
        "image_beta": np.zeros(CO, np.float32),
        "image_mean": rng.standard_normal(CO).astype(np.float32) * 0.1,
        "image_var": rng.uniform(0.5, 1.5, CO).astype(np.float32),
        "modality_weights": np.ones(2, np.float32),
    }
    out = kernel(**inputs)
    print("kernel out:", out.shape, out.dtype, float(np.abs(out).mean()))


# revision 28
# speedup vs baseline: 1.2543x; 1.2543x over previous
"""CrossAttentionFusion kernel for Trainium2 (8 NeuronCores, Bass/Tile).

Computation (matches the reference nn.Module):
  image_proj = relu(BN(1x1conv(image_features, image_w)))   # (B,128,H,W)
  lidar_proj = relu(BN(1x1conv(lidar_features, lidar_w)))   # (B,128,H,W)
  per (batch, 2048-pixel chunk): q = image_proj, k = v = lidar_proj
  attn_out = softmax(q k^T / sqrt(128)) @ k
  out = w0 * image_proj + w1 * attn_out,  w = softmax(modality_weights)

Sharding: the 16 independent (batch, chunk) attention problems are
distributed 2-per-core across 8 cores; each core also computes the
projections for its own pixels.  Host gathers the 8 outputs.

Per-core kernel layout notes (bf16 pipeline):
  - All matmul operands are bf16; PSUM accumulation stays fp32.  Host
    converts inputs to bf16 and folds the BN scale into the weights, so
    the projection epilogue is a single DVE tensor_scalar:
    max(psum + bias, 0) -> bf16.
  - Matmuls are 1024 wide (output spans two PSUM banks), halving the
    instruction + LDWEIGHTS count vs 512-wide halves.
  - Scores are computed k-major: ps[kslice, q]; exp() on the scalar
    engine writes bf16 (the scalar engine is the pace-setter: ~1.3us
    per 1024-wide exp, 64 of them).
  - AV uses transposed-K tiles: po[c, q] += Kpix_i^T @ ET_i, lagging
    LOOKAHEAD slices behind the score stream; the slice loop runs
    globally across both q-blocks so the PE never drains at block
    boundaries.
  - softmax denominator: binary-tree bf16 adds of the 16 ET tiles on
    the vector engine (S), then (ones/w1)^T @ S broadcast-sums across
    partitions on the PE; linv = 1/pl then already carries w1.
  - w0 is folded into the image BN affine (relu(w0*x) = w0*relu(x));
    the exp scale compensates with 1/(w0*sqrt(C)).
  - Output written per-qb as bf16; host casts back to fp32.
"""

import math
import os
import sys
from contextlib import ExitStack

import ml_dtypes
import numpy as np

sys.path.insert(0, "/opt/trn_rl_repo")

import concourse.bass as bass  # noqa: E402
import concourse.tile as tile  # noqa: E402
from concourse import bacc, bass_isa, mybir  # noqa: E402
from concourse.bass import ds, ts  # noqa: E402
from concourse.bass_utils import run_bass_kernel_spmd  # noqa: E402

F32 = mybir.dt.float32
BF16 = mybir.dt.bfloat16
NPBF16 = ml_dtypes.bfloat16

B, CL, CI, CO = 2, 256, 512, 128
H = W = 128
P = H * W                    # 16384 pixels per batch
CHUNK = 2048                 # attention chunk (pixels)
NCH = P // CHUNK             # 8 chunks per batch
NCORES = 8
UPC = (B * NCH) // NCORES    # units (b,chunk) per core = 2
EPS = 1e-5
QB = 512                     # q-block width (one 1-bank PSUM matmul)
NQB = CHUNK // QB            # 4
KSL = CHUNK // 128           # 16 k-pixel slices per chunk
NSL = NQB * KSL              # 32 (qb, slice) score tiles per unit
NCI_IMG = CI // 128          # 4 contraction slices for image proj
NCI_LID = CL // 128          # 2 for lidar proj

_PROGRAM = None              # compiled Bass program, built once per process
LAST_RESULTS = None          # BassKernelResults of the last kernel() call


def _build_program():
    nc = bacc.Bacc("TRN2", target_bir_lowering=False, debug=False,
                   num_devices=NCORES)

    # Per-core DRAM inputs (pre-sharded, bf16 on host, constants packed so
    # the startup path is few DMA issues).
    ximg = nc.dram_tensor("ximg", [UPC, NCI_IMG, 128, CHUNK], BF16,
                          kind="ExternalInput").ap()
    xlid = nc.dram_tensor("xlid", [UPC, NCI_LID, 128, CHUNK], BF16,
                          kind="ExternalInput").ap()
    wimg = nc.dram_tensor("wimg", [128, NCI_IMG * CO], BF16,
                          kind="ExternalInput").ap()
    wlid = nc.dram_tensor("wlid", [128, NCI_LID * CO], BF16,
                          kind="ExternalInput").ap()
    # columns: img_bias, lid_bias, exp_scale, w1
    scal = nc.dram_tensor("scal", [128, 4], F32, kind="ExternalInput").ap()
    # columns 0:128 identity, 128:256 ones/w1
    idon = nc.dram_tensor("idon", [128, 256], BF16, kind="ExternalInput").ap()
    y = nc.dram_tensor("y", [UPC, NQB, CO, QB], BF16, kind="ExternalOutput").ap()

    with tile.TileContext(nc) as tc, ExitStack() as ctx:
        const = ctx.enter_context(tc.tile_pool(name="const", bufs=1))
        xi_pool = ctx.enter_context(tc.tile_pool(name="xi", bufs=UPC * NCI_IMG))
        xl_pool = ctx.enter_context(tc.tile_pool(name="xl", bufs=UPC * NCI_LID))
        proj_pool = ctx.enter_context(tc.tile_pool(name="proj", bufs=4))
        kp_pool = ctx.enter_context(tc.tile_pool(name="kp", bufs=4))
        et_pool = ctx.enter_context(tc.tile_pool(name="et", bufs=6))
        tree_pool = ctx.enter_context(tc.tile_pool(name="tree", bufs=10))
        misc_pool = ctx.enter_context(tc.tile_pool(name="misc", bufs=3))
        res_pool = ctx.enter_context(tc.tile_pool(name="res", bufs=3))
        # PSUM (8 banks of [128, 2KB]):
        #   mm 2x[128,1536]f32 (6 banks) - score groups of 3 k-slices,
        #     double buffered, so exp() runs 1536 wide; also borrowed for
        #     the projection groups ([128,512] each)
        #   av 2x[128,512]f32 (2 banks) - AV accumulation + the lb tile,
        #     plus the transpose/img-tail projection scratch ([128,1024]bf16
        #     / [128,512]f32 both fit a bank)
        mm_psum = ctx.enter_context(tc.tile_pool(name="mmps", bufs=2, space="PSUM"))
        av_psum = ctx.enter_context(tc.tile_pool(name="avps", bufs=2, space="PSUM"))

        # ---- startup-critical DMAs on the sync queue, in dependency order
        # for the first projection group (lidar first); the rest spread over
        # the scalar and gpsimd queues, which are otherwise idle here ----
        xl = {0: [xl_pool.tile([128, CHUNK], BF16, name=f"xl_0_{ci}", tag="xl")
                  for ci in range(NCI_LID)]}
        xi = {0: [xi_pool.tile([128, CHUNK], BF16, name=f"xi_0_{ci}", tag="xi")
                  for ci in range(NCI_IMG)]}
        for ci in range(NCI_LID):
            nc.sync.dma_start(xl[0][ci][:, ds(0, 1024)], xlid[0, ci, :, ds(0, 1024)])
        wlid_t = const.tile([128, NCI_LID * CO], BF16)
        nc.sync.dma_start(wlid_t[:], wlid)
        scal_t = const.tile([128, 4], F32)
        nc.sync.dma_start(scal_t[:], scal)
        img_b = scal_t[:, ds(0, 1)]
        lid_b = scal_t[:, ds(1, 1)]
        esc = scal_t[:, ds(2, 1)]
        for ci in range(NCI_LID):
            nc.scalar.dma_start(xl[0][ci][:, ds(1024, 1024)],
                                xlid[0, ci, :, ds(1024, 1024)])
        wimg_t = const.tile([128, NCI_IMG * CO], BF16)
        nc.gpsimd.dma_start(wimg_t[:], wimg)
        idon_t = const.tile([128, 256], BF16)
        nc.gpsimd.dma_start(idon_t[:], idon)
        ident_t = idon_t[:, ds(0, 128)]
        ones_t = idon_t[:, ds(128, 128)]
        # pre-trigger the activation table load while DMAs are in flight
        warm = const.tile([128, 4], F32)
        nc.scalar.activation(warm[:], scal_t[:],
                             mybir.ActivationFunctionType.Relu)
        for ci in range(NCI_IMG):
            nc.gpsimd.dma_start(xi[0][ci][:, ds(0, 1024)],
                                ximg[0, ci, :, ds(0, 1024)])
        for ci in range(NCI_IMG):
            nc.gpsimd.dma_start(xi[0][ci][:, ds(1024, 1024)],
                                ximg[0, ci, :, ds(1024, 1024)])
        for u in range(1, UPC):
            xl[u] = [xl_pool.tile([128, CHUNK], BF16, name=f"xl_{u}_{ci}",
                                  tag="xl") for ci in range(NCI_LID)]
            xi[u] = [xi_pool.tile([128, CHUNK], BF16, name=f"xi_{u}_{ci}",
                                  tag="xi") for ci in range(NCI_IMG)]
            for ci in range(NCI_LID):
                nc.gpsimd.dma_start(xl[u][ci][:], xlid[u, ci])
            for ci in range(NCI_IMG):
                nc.gpsimd.dma_start(xi[u][ci][:], ximg[u, ci])

        qT = {}
        kT = {}
        kpw = {}

        def proj_group(u, dst, w_t, nci, xsrc, q0, bias, on_act):
            """One [128, QB] projection group: matmul accumulate + relu
            (PSUM always from the score pool; insertions are emitted in
            parity-preserving pairs).  The epilogue runs on ACT where it
            fills idle time, or on the DVE when slotted into the exp
            stream."""
            ps = pool_tile = mm_psum.tile(
                [128, QB], F32, name=f"pj_{u}_{dst.tensor.name}_{q0}",
                tag="ps")
            for ci in range(nci):
                nc.tensor.matmul(ps[:], w_t[:, ts(ci, CO)],
                                 xsrc[ci][:, ds(q0, QB)],
                                 start=(ci == 0), stop=(ci == nci - 1))
            if on_act:
                nc.scalar.activation(dst[:, ds(q0, QB)], ps[:],
                                     mybir.ActivationFunctionType.Relu,
                                     bias=bias)
            else:
                nc.vector.tensor_scalar(dst[:, ds(q0, QB)], ps[:], bias, 0.0,
                                        op0=mybir.AluOpType.add,
                                        op1=mybir.AluOpType.max)

        def emit_proj_part1(u):
            """Lidar projection (all of kT) and the first half of qT, relu
            on the scalar engine (idle in this phase)."""
            qT[u] = proj_pool.tile([128, CHUNK], BF16, name=f"qT_{u}", tag="qT")
            kT[u] = proj_pool.tile([128, CHUNK], BF16, name=f"kT_{u}", tag="kT")
            kpw[u] = []
            for q0 in range(0, CHUNK, QB):
                proj_group(u, kT[u], wlid_t, NCI_LID, xl[u], q0, lid_b, True)
            for q0 in range(0, CHUNK // 2, QB):
                proj_group(u, qT[u], wimg_t, NCI_IMG, xi[u], q0, img_b, True)

        def emit_transpose_group(u, g):
            """One 8-slice K transpose group, PSUM borrowed from the (still
            idle) AV pool; rides the early-stream PE slack."""
            pt = av_psum.tile([128, 8 * 128], BF16, name=f"pt_{u}_{g}",
                              tag="av")
            last_w[0] = None
            for k in range(8):
                nc.tensor.transpose(pt[:, ts(k, 128)],
                                    kT[u][:, ts(g * 8 + k, 128)], ident_t)
            last_w[0] = None
            kpt = kp_pool.tile([128, 8 * 128], BF16, name=f"kp_{u}_{g}",
                               tag="kp")
            nc.vector.tensor_copy(kpt[:], pt[:])
            kpw[u].append(kpt)

        last_w = [None]

        def mm_dedup(out, lhsT, rhs, start, stop):
            key = (lhsT.tensor.name, lhsT.offset)
            inst = nc.tensor.matmul(out, lhsT, rhs, start=start, stop=stop)
            if last_w[0] == key:
                inst.ins.ldweights = True
            last_w[0] = key
            return inst

        # group schedule per unit: per q-block (QB wide), the 16 k-slices
        # are exp'd in groups of 3 (5x1536-wide + 1x512-wide)
        GRPS = []
        for qb in range(NQB):
            for s0 in range(0, KSL, 3):
                GRPS.append((qb, list(range(s0, min(s0 + 3, KSL)))))
        NG = len(GRPS)                     # 24 groups per unit
        GLA = 2                            # AV lags 2 exp groups

        emit_proj_part1(0)
        for u in range(UPC):
            ets = {}                       # (qb, slice) -> et view
            pos = {}
            lvl1 = {}
            lvl2 = {}
            lvl3 = {}
            done_tree = set()

            def tree_step(qb):
                """Emit any newly-possible S-tree adds for this q-block.
                Partial sums kept 4-way (t3[0-7], t2[8-11], t1[12-13],
                t1[14-15]) so only one DVE add separates the last exp from
                the lb matmul."""
                for p in range(KSL // 2):
                    i0, i1 = 2 * p, 2 * p + 1
                    if (qb, p) in done_tree or (qb, i0) not in ets \
                            or (qb, i1) not in ets:
                        continue
                    done_tree.add((qb, p))
                    t = tree_pool.tile([128, QB], BF16,
                                       name=f"t1_{u}_{qb}_{p}", tag="t1")
                    nc.vector.tensor_add(t[:], ets[(qb, i0)][:],
                                         ets[(qb, i1)][:])
                    lvl1[(qb, p)] = t
                    if p % 2 == 1 and p < 6:
                        t2 = tree_pool.tile([128, QB], BF16,
                                            name=f"t2_{u}_{qb}_{p}", tag="t1")
                        nc.vector.tensor_add(t2[:], lvl1[(qb, p - 1)][:], t[:])
                        lvl2[(qb, p // 2)] = t2
                        if p == 3:
                            t3 = tree_pool.tile([128, QB], BF16,
                                                name=f"t3_{u}_{qb}", tag="t1")
                            nc.vector.tensor_add(t3[:], lvl2[(qb, 0)][:],
                                                 t2[:])
                            lvl3[qb] = t3

            def qb_epilogue(qb):
                """Denominator broadcast ((ones/w1)^T @ partial S tiles) into
                an AV-pool tile, then reciprocal + blend + store."""
                pl = av_psum.tile([128, QB], F32, name=f"pl_{u}_{qb}",
                                  tag="av")
                parts = [lvl3[qb], lvl2[(qb, 2)], lvl1[(qb, 6)],
                         lvl1[(qb, 7)]]
                for pi, part in enumerate(parts):
                    mm_dedup(pl[:], ones_t, part[:],
                             start=(pi == 0), stop=(pi == len(parts) - 1))
                linv = misc_pool.tile([128, QB], F32, name=f"linv_{u}_{qb}",
                                      tag="linv")
                tmp = misc_pool.tile([128, QB], F32, name=f"tmp_{u}_{qb}",
                                     tag="tmp")
                res = res_pool.tile([128, QB], BF16, name=f"res_{u}_{qb}",
                                    tag="res")
                nc.vector.reciprocal_approx_fast(linv[:], pl[:])
                nc.vector.tensor_mul(tmp[:], pos[qb][:], linv[:])
                nc.vector.tensor_add(res[:], tmp[:], qT[u][:, ts(qb, QB)])
                nc.sync.dma_start(y[u, qb], res[:])

            for G in range(NG + GLA):
                if G < NG:
                    qb, slices = GRPS[G]
                    w = len(slices) * QB
                    ps = mm_psum.tile([128, 3 * QB], F32,
                                      name=f"sc_{u}_{G}", tag="ps")
                    for si, s in enumerate(slices):
                        mm_dedup(ps[:, ts(si, QB)], kT[u][:, ts(s, 128)],
                                 qT[u][:, ts(qb, QB)], start=True, stop=True)
                    et = et_pool.tile([128, 3 * QB], BF16,
                                      name=f"et_{u}_{G}", tag="et")
                    nc.scalar.activation(et[:, ds(0, w)], ps[:, ds(0, w)],
                                         mybir.ActivationFunctionType.Exp,
                                         scale=esc)
                    for si, s in enumerate(slices):
                        ets[(qb, s)] = et[:, ts(si, QB)]
                    tree_step(qb)
                    # early-stream insertions: transposes + tail of the
                    # image projection (relu on DVE, PSUM from the AV pool)
                    if 1 <= G <= 2:
                        emit_transpose_group(u, G - 1)
                    elif G == 3:
                        # both image-tail groups as one parity-preserving
                        # pair of score-pool allocations
                        proj_group(u, qT[u], wimg_t, NCI_IMG, xi[u],
                                   CHUNK // 2, img_b, False)
                        proj_group(u, qT[u], wimg_t, NCI_IMG, xi[u],
                                   CHUNK // 2 + QB, img_b, False)
                else:
                    if G == NG and u + 1 < UPC:
                        emit_proj_part1(u + 1)
                J = G - GLA
                if 0 <= J < NG:
                    qbj, jslices = GRPS[J]
                    if J % 6 == 0:
                        pos[qbj] = av_psum.tile([128, QB], F32,
                                                name=f"po_{u}_{qbj}",
                                                tag="av")
                    for s in jslices:
                        kslice = kpw[u][s // 8][:, ts(s % 8, 128)]
                        mm_dedup(pos[qbj][:], kslice, ets[(qbj, s)][:],
                                 start=(s == 0), stop=(s == KSL - 1))
                    if jslices[-1] == KSL - 1:
                        qb_epilogue(qbj)

    nc.compile()
    return nc


def _shard_inputs(inputs):
    """Build the 8 per-core input maps from the full input dict."""
    mw = np.asarray(inputs["modality_weights"], np.float64)
    e = np.exp(mw - mw.max())
    w = (e / e.sum()).astype(np.float64)
    w0, w1 = float(w[0]), float(w[1])

    def bn_fold(gamma, beta, mean, var, mul):
        g = np.asarray(gamma, np.float64)
        b = np.asarray(beta, np.float64)
        m = np.asarray(mean, np.float64)
        v = np.asarray(var, np.float64)
        scale = g / np.sqrt(v + EPS) * mul
        bias = (b - m * g / np.sqrt(v + EPS)) * mul
        return scale, bias.astype(np.float32)

    i_s, i_b = bn_fold(inputs["image_gamma"], inputs["image_beta"],
                       inputs["image_mean"], inputs["image_var"], w0)
    l_s, l_b = bn_fold(inputs["lidar_gamma"], inputs["lidar_beta"],
                       inputs["lidar_mean"], inputs["lidar_var"], 1.0)

    # weight slices with the BN scale folded in, packed [cin(128), ci*CO]
    wi = (np.asarray(inputs["image_w"], np.float64) * i_s[:, None]).T.reshape(
        NCI_IMG, 128, CO).transpose(1, 0, 2).reshape(128, NCI_IMG * CO)
    wi = np.ascontiguousarray(wi).astype(NPBF16)
    wl = (np.asarray(inputs["lidar_w"], np.float64) * l_s[:, None]).T.reshape(
        NCI_LID, 128, CO).transpose(1, 0, 2).reshape(128, NCI_LID * CO)
    wl = np.ascontiguousarray(wl).astype(NPBF16)

    scal = np.zeros((128, 4), np.float32)
    scal[:, 0] = i_b
    scal[:, 1] = l_b
    scal[:, 2] = 1.0 / (w0 * math.sqrt(CO))
    scal[:, 3] = w1
    idon = np.zeros((128, 256), np.float32)
    idon[:, :128] = np.eye(128, dtype=np.float32)
    # ones carry 1/w1 so linv = 1/pl = w1/denominator
    idon[:, 128:] = 1.0 / w1
    idon = idon.astype(NPBF16)

    # full features reshaped to (B, nchunks, C, 2048), bf16
    img = np.asarray(inputs["image_features"], np.float32).reshape(
        B, CI, NCH, CHUNK).astype(NPBF16)
    lid = np.asarray(inputs["lidar_features"], np.float32).reshape(
        B, CL, NCH, CHUNK).astype(NPBF16)

    in_maps = []
    for core in range(NCORES):
        ximg = np.empty((UPC, NCI_IMG, 128, CHUNK), NPBF16)
        xlid = np.empty((UPC, NCI_LID, 128, CHUNK), NPBF16)
        for ul in range(UPC):
            un = core * UPC + ul
            b, c = un // NCH, un % NCH
            ximg[ul] = img[b, :, c, :].reshape(NCI_IMG, 128, CHUNK)
            xlid[ul] = lid[b, :, c, :].reshape(NCI_LID, 128, CHUNK)
        in_maps.append({
            "ximg": ximg, "xlid": xlid, "wimg": wi, "wlid": wl,
            "scal": scal, "idon": idon,
        })
    return in_maps


def kernel(**inputs) -> np.ndarray:
    global _PROGRAM, LAST_RESULTS
    if _PROGRAM is None:
        _PROGRAM = _build_program()
    nc = _PROGRAM

    in_maps = _shard_inputs(inputs)
    trace = os.environ.get("BASS_KERNEL_TRACE", "0") == "1"
    tmpdir = os.environ.get("BASS_KERNEL_TRACE_DIR") or None
    if tmpdir:
        os.makedirs(tmpdir, exist_ok=True)
    results = run_bass_kernel_spmd(nc, in_maps, core_ids=list(range(NCORES)),
                                   trace=trace, tmpdir=tmpdir)
    LAST_RESULTS = results

    out = np.empty((B, CO, H, W), np.float32)
    outv = out.reshape(B, CO, NCH, NQB, QB)
    for core in range(NCORES):
        yc = np.asarray(results.results[core]["y"], dtype=np.float32)
        for ul in range(UPC):
            un = core * UPC + ul
            b, c = un // NCH, un % NCH
            outv[b, :, c, :, :] = yc[ul].transpose(1, 0, 2)
    return out


if __name__ == "__main__":
    rng = np.random.default_rng(0)
    inputs = {
        "lidar_features": rng.standard_normal((B, CL, H, W), np.float32),
        "image_features": rng.standard_normal((B, CI, H, W), np.float32),
        "lidar_w": rng.standard_normal((CO, CL), np.float32) * np.sqrt(2.0 / CO),
        "lidar_gamma": np.ones(CO, np.float32),
        "lidar_beta": np.zeros(CO, np.float32),
        "lidar_mean": rng.standard_normal(CO).astype(np.float32) * 0.1,
        "lidar_var": rng.uniform(0.5, 1.5, CO).astype(np.float32),
        "image_w": rng.standard_normal((CO, CI), np.float32) * np.sqrt(2.0 / CO),
        "image_gamma": np.ones(CO, np.float32),
        "image_beta": np.zeros(CO, np.float32),
        "image_mean": rng.standard_normal(CO).astype(np.float32) * 0.1,
        "image_var": rng.uniform(0.5, 1.5, CO).astype(np.float32),
        "modality_weights": np.ones(2, np.float32),
    }
    out = kernel(**inputs)
    print("kernel out:", out.shape, out.dtype, float(np.abs(out).mean()))


# revision 29
# speedup vs baseline: 1.2855x; 1.0249x over previous
"""CrossAttentionFusion kernel for Trainium2 (8 NeuronCores, Bass/Tile).

Computation (matches the reference nn.Module):
  image_proj = relu(BN(1x1conv(image_features, image_w)))   # (B,128,H,W)
  lidar_proj = relu(BN(1x1conv(lidar_features, lidar_w)))   # (B,128,H,W)
  per (batch, 2048-pixel chunk): q = image_proj, k = v = lidar_proj
  attn_out = softmax(q k^T / sqrt(128)) @ k
  out = w0 * image_proj + w1 * attn_out,  w = softmax(modality_weights)

Sharding: the 16 independent (batch, chunk) attention problems are
distributed 2-per-core across 8 cores; each core also computes the
projections for its own pixels.  Host gathers the 8 outputs.

Per-core kernel layout notes (bf16 pipeline):
  - All matmul operands are bf16; PSUM accumulation stays fp32.  Host
    converts inputs to bf16 and folds the BN scale into the weights, so
    the projection epilogue is a single DVE tensor_scalar:
    max(psum + bias, 0) -> bf16.
  - Matmuls are 1024 wide (output spans two PSUM banks), halving the
    instruction + LDWEIGHTS count vs 512-wide halves.
  - Scores are computed k-major: ps[kslice, q]; exp() on the scalar
    engine writes bf16 (the scalar engine is the pace-setter: ~1.3us
    per 1024-wide exp, 64 of them).
  - AV uses transposed-K tiles: po[c, q] += Kpix_i^T @ ET_i, lagging
    LOOKAHEAD slices behind the score stream; the slice loop runs
    globally across both q-blocks so the PE never drains at block
    boundaries.
  - softmax denominator: binary-tree bf16 adds of the 16 ET tiles on
    the vector engine (S), then (ones/w1)^T @ S broadcast-sums across
    partitions on the PE; linv = 1/pl then already carries w1.
  - w0 is folded into the image BN affine (relu(w0*x) = w0*relu(x));
    the exp scale compensates with 1/(w0*sqrt(C)).
  - Output written per-qb as bf16; host casts back to fp32.
"""

import math
import os
import sys
from contextlib import ExitStack

import ml_dtypes
import numpy as np

sys.path.insert(0, "/opt/trn_rl_repo")

import concourse.bass as bass  # noqa: E402
import concourse.tile as tile  # noqa: E402
from concourse import bacc, bass_isa, mybir  # noqa: E402
from concourse.bass import ds, ts  # noqa: E402
from concourse.bass_utils import run_bass_kernel_spmd  # noqa: E402

F32 = mybir.dt.float32
BF16 = mybir.dt.bfloat16
NPBF16 = ml_dtypes.bfloat16

B, CL, CI, CO = 2, 256, 512, 128
H = W = 128
P = H * W                    # 16384 pixels per batch
CHUNK = 2048                 # attention chunk (pixels)
NCH = P // CHUNK             # 8 chunks per batch
NCORES = 8
UPC = (B * NCH) // NCORES    # units (b,chunk) per core = 2
EPS = 1e-5
QB = 512                     # q-block width (one 1-bank PSUM matmul)
NQB = CHUNK // QB            # 4
KSL = CHUNK // 128           # 16 k-pixel slices per chunk
NSL = NQB * KSL              # 32 (qb, slice) score tiles per unit
NCI_IMG = CI // 128          # 4 contraction slices for image proj
NCI_LID = CL // 128          # 2 for lidar proj

_PROGRAM = None              # compiled Bass program, built once per process
LAST_RESULTS = None          # BassKernelResults of the last kernel() call


def _build_program():
    nc = bacc.Bacc("TRN2", target_bir_lowering=False, debug=False,
                   num_devices=NCORES)

    # Per-core DRAM inputs (pre-sharded, bf16 on host, constants packed so
    # the startup path is few DMA issues).
    ximg = nc.dram_tensor("ximg", [UPC, NCI_IMG, 128, CHUNK], BF16,
                          kind="ExternalInput").ap()
    xlid = nc.dram_tensor("xlid", [UPC, NCI_LID, 128, CHUNK], BF16,
                          kind="ExternalInput").ap()
    wimg = nc.dram_tensor("wimg", [128, NCI_IMG * CO], BF16,
                          kind="ExternalInput").ap()
    wlid = nc.dram_tensor("wlid", [128, NCI_LID * CO], BF16,
                          kind="ExternalInput").ap()
    # columns: img_bias, lid_bias, exp_scale, w1
    scal = nc.dram_tensor("scal", [128, 4], F32, kind="ExternalInput").ap()
    # columns 0:128 identity, 128:256 ones/w1
    idon = nc.dram_tensor("idon", [128, 256], BF16, kind="ExternalInput").ap()
    y = nc.dram_tensor("y", [UPC, NQB, CO, QB], BF16, kind="ExternalOutput").ap()

    with tile.TileContext(nc) as tc, ExitStack() as ctx:
        const = ctx.enter_context(tc.tile_pool(name="const", bufs=1))
        xi_pool = ctx.enter_context(tc.tile_pool(name="xi", bufs=UPC * NCI_IMG))
        xl_pool = ctx.enter_context(tc.tile_pool(name="xl", bufs=UPC * NCI_LID))
        proj_pool = ctx.enter_context(tc.tile_pool(name="proj", bufs=4))
        kp_pool = ctx.enter_context(tc.tile_pool(name="kp", bufs=4))
        et_pool = ctx.enter_context(tc.tile_pool(name="et", bufs=6))
        tree_pool = ctx.enter_context(tc.tile_pool(name="tree", bufs=10))
        misc_pool = ctx.enter_context(tc.tile_pool(name="misc", bufs=3))
        res_pool = ctx.enter_context(tc.tile_pool(name="res", bufs=3))
        # PSUM (8 banks of [128, 2KB]):
        #   mm 2x[128,1536]f32 (6 banks) - score groups of 3 k-slices,
        #     double buffered, so exp() runs 1536 wide; also borrowed for
        #     the projection groups ([128,512] each)
        #   av 2x[128,512]f32 (2 banks) - AV accumulation + the lb tile,
        #     plus the transpose/img-tail projection scratch ([128,1024]bf16
        #     / [128,512]f32 both fit a bank)
        mm_psum = ctx.enter_context(tc.tile_pool(name="mmps", bufs=2, space="PSUM"))
        av_psum = ctx.enter_context(tc.tile_pool(name="avps", bufs=2, space="PSUM"))

        # ---- startup-critical DMAs on the sync queue, in dependency order
        # for the first projection group (lidar first); the rest spread over
        # the scalar and gpsimd queues, which are otherwise idle here ----
        xl = {0: [xl_pool.tile([128, CHUNK], BF16, name=f"xl_0_{ci}", tag="xl")
                  for ci in range(NCI_LID)]}
        xi = {0: [xi_pool.tile([128, CHUNK], BF16, name=f"xi_0_{ci}", tag="xi")
                  for ci in range(NCI_IMG)]}
        for ci in range(NCI_LID):
            nc.sync.dma_start(xl[0][ci][:, ds(0, 1024)], xlid[0, ci, :, ds(0, 1024)])
        wlid_t = const.tile([128, NCI_LID * CO], BF16)
        nc.sync.dma_start(wlid_t[:], wlid)
        scal_t = const.tile([128, 4], F32)
        nc.sync.dma_start(scal_t[:], scal)
        img_b = scal_t[:, ds(0, 1)]
        lid_b = scal_t[:, ds(1, 1)]
        esc = scal_t[:, ds(2, 1)]
        for ci in range(NCI_LID):
            nc.scalar.dma_start(xl[0][ci][:, ds(1024, 1024)],
                                xlid[0, ci, :, ds(1024, 1024)])
        wimg_t = const.tile([128, NCI_IMG * CO], BF16)
        nc.gpsimd.dma_start(wimg_t[:], wimg)
        idon_t = const.tile([128, 256], BF16)
        nc.gpsimd.dma_start(idon_t[:], idon)
        ident_t = idon_t[:, ds(0, 128)]
        ones_t = idon_t[:, ds(128, 128)]
        # pre-trigger the activation table load while DMAs are in flight
        warm = const.tile([128, 4], F32)
        nc.scalar.activation(warm[:], scal_t[:],
                             mybir.ActivationFunctionType.Relu)
        for ci in range(NCI_IMG):
            nc.gpsimd.dma_start(xi[0][ci][:, ds(0, 1024)],
                                ximg[0, ci, :, ds(0, 1024)])
        for ci in range(NCI_IMG):
            nc.gpsimd.dma_start(xi[0][ci][:, ds(1024, 1024)],
                                ximg[0, ci, :, ds(1024, 1024)])
        for u in range(1, UPC):
            xl[u] = [xl_pool.tile([128, CHUNK], BF16, name=f"xl_{u}_{ci}",
                                  tag="xl") for ci in range(NCI_LID)]
            xi[u] = [xi_pool.tile([128, CHUNK], BF16, name=f"xi_{u}_{ci}",
                                  tag="xi") for ci in range(NCI_IMG)]
            for ci in range(NCI_LID):
                nc.gpsimd.dma_start(xl[u][ci][:], xlid[u, ci])
            for ci in range(NCI_IMG):
                nc.gpsimd.dma_start(xi[u][ci][:], ximg[u, ci])

        qT = {}
        kT = {}
        kpw = {}

        def proj_group(u, dst, w_t, nci, xsrc, q0, bias, on_act):
            """One [128, QB] projection group: matmul accumulate + relu
            (PSUM always from the score pool; insertions are emitted in
            parity-preserving pairs).  The epilogue runs on ACT where it
            fills idle time, or on the DVE when slotted into the exp
            stream."""
            pool, tag = (mm_psum, "ps") if on_act else (av_psum, "av")
            ps = pool.tile([128, QB], F32,
                           name=f"pj_{u}_{dst.tensor.name}_{q0}", tag=tag)
            for ci in range(nci):
                nc.tensor.matmul(ps[:], w_t[:, ts(ci, CO)],
                                 xsrc[ci][:, ds(q0, QB)],
                                 start=(ci == 0), stop=(ci == nci - 1))
            if on_act:
                nc.scalar.activation(dst[:, ds(q0, QB)], ps[:],
                                     mybir.ActivationFunctionType.Relu,
                                     bias=bias)
            else:
                nc.vector.tensor_scalar(dst[:, ds(q0, QB)], ps[:], bias, 0.0,
                                        op0=mybir.AluOpType.add,
                                        op1=mybir.AluOpType.max)

        def emit_proj_part1(u):
            """Lidar projection (all of kT) and the first half of qT, relu
            on the scalar engine (idle in this phase)."""
            qT[u] = proj_pool.tile([128, CHUNK], BF16, name=f"qT_{u}", tag="qT")
            kT[u] = proj_pool.tile([128, CHUNK], BF16, name=f"kT_{u}", tag="kT")
            kpw[u] = []
            for q0 in range(0, CHUNK, QB):
                proj_group(u, kT[u], wlid_t, NCI_LID, xl[u], q0, lid_b, True)
            for q0 in range(0, CHUNK // 2, QB):
                proj_group(u, qT[u], wimg_t, NCI_IMG, xi[u], q0, img_b, True)

        def emit_transpose_group(u, g):
            """One 8-slice K transpose group, PSUM borrowed from the (still
            idle) AV pool; rides the early-stream PE slack."""
            pt = av_psum.tile([128, 8 * 128], BF16, name=f"pt_{u}_{g}",
                              tag="av")
            last_w[0] = None
            for k in range(8):
                nc.tensor.transpose(pt[:, ts(k, 128)],
                                    kT[u][:, ts(g * 8 + k, 128)], ident_t)
            last_w[0] = None
            kpt = kp_pool.tile([128, 8 * 128], BF16, name=f"kp_{u}_{g}",
                               tag="kp")
            nc.vector.tensor_copy(kpt[:], pt[:])
            kpw[u].append(kpt)

        last_w = [None]

        def mm_dedup(out, lhsT, rhs, start, stop):
            key = (lhsT.tensor.name, lhsT.offset)
            inst = nc.tensor.matmul(out, lhsT, rhs, start=start, stop=stop)
            if last_w[0] == key:
                inst.ins.ldweights = True
            last_w[0] = key
            return inst

        # group schedule per unit: per q-block (QB wide), the 16 k-slices
        # are exp'd in groups of 3 (5x1536-wide + 1x512-wide)
        GRPS = []
        for qb in range(NQB):
            for s0 in range(0, KSL, 3):
                GRPS.append((qb, list(range(s0, min(s0 + 3, KSL)))))
        NG = len(GRPS)                     # 24 groups per unit
        GLA = 4                            # AV lags 4 exp groups

        emit_proj_part1(0)
        for u in range(UPC):
            ets = {}                       # (qb, slice) -> et view
            pos = {}
            lvl1 = {}
            lvl2 = {}
            lvl3 = {}
            done_tree = set()

            def tree_step(qb):
                """Emit any newly-possible S-tree adds for this q-block.
                Partial sums kept 4-way (t3[0-7], t2[8-11], t1[12-13],
                t1[14-15]) so only one DVE add separates the last exp from
                the lb matmul."""
                for p in range(KSL // 2):
                    i0, i1 = 2 * p, 2 * p + 1
                    if (qb, p) in done_tree or (qb, i0) not in ets \
                            or (qb, i1) not in ets:
                        continue
                    done_tree.add((qb, p))
                    t = tree_pool.tile([128, QB], BF16,
                                       name=f"t1_{u}_{qb}_{p}", tag="t1")
                    nc.vector.tensor_add(t[:], ets[(qb, i0)][:],
                                         ets[(qb, i1)][:])
                    lvl1[(qb, p)] = t
                    if p % 2 == 1 and p < 6:
                        t2 = tree_pool.tile([128, QB], BF16,
                                            name=f"t2_{u}_{qb}_{p}", tag="t1")
                        nc.vector.tensor_add(t2[:], lvl1[(qb, p - 1)][:], t[:])
                        lvl2[(qb, p // 2)] = t2
                        if p == 3:
                            t3 = tree_pool.tile([128, QB], BF16,
                                                name=f"t3_{u}_{qb}", tag="t1")
                            nc.vector.tensor_add(t3[:], lvl2[(qb, 0)][:],
                                                 t2[:])
                            lvl3[qb] = t3

            def qb_epilogue(qb):
                """Denominator broadcast ((ones/w1)^T @ partial S tiles) into
                an AV-pool tile, then reciprocal + blend + store."""
                pl = av_psum.tile([128, QB], F32, name=f"pl_{u}_{qb}",
                                  tag="av")
                parts = [lvl3[qb], lvl2[(qb, 2)], lvl1[(qb, 6)],
                         lvl1[(qb, 7)]]
                for pi, part in enumerate(parts):
                    mm_dedup(pl[:], ones_t, part[:],
                             start=(pi == 0), stop=(pi == len(parts) - 1))
                linv = misc_pool.tile([128, QB], F32, name=f"linv_{u}_{qb}",
                                      tag="linv")
                tmp = misc_pool.tile([128, QB], F32, name=f"tmp_{u}_{qb}",
                                     tag="tmp")
                res = res_pool.tile([128, QB], BF16, name=f"res_{u}_{qb}",
                                    tag="res")
                nc.vector.reciprocal_approx_fast(linv[:], pl[:])
                nc.vector.tensor_mul(tmp[:], pos[qb][:], linv[:])
                nc.vector.tensor_add(res[:], tmp[:], qT[u][:, ts(qb, QB)])
                nc.sync.dma_start(y[u, qb], res[:])

            for G in range(NG + GLA):
                if G < NG:
                    qb, slices = GRPS[G]
                    w = len(slices) * QB
                    ps = mm_psum.tile([128, 3 * QB], F32,
                                      name=f"sc_{u}_{G}", tag="ps")
                    for si, s in enumerate(slices):
                        mm_dedup(ps[:, ts(si, QB)], kT[u][:, ts(s, 128)],
                                 qT[u][:, ts(qb, QB)], start=True, stop=True)
                    et = et_pool.tile([128, 3 * QB], BF16,
                                      name=f"et_{u}_{G}", tag="et")
                    nc.scalar.activation(et[:, ds(0, w)], ps[:, ds(0, w)],
                                         mybir.ActivationFunctionType.Exp,
                                         scale=esc)
                    for si, s in enumerate(slices):
                        ets[(qb, s)] = et[:, ts(si, QB)]
                    tree_step(qb)
                    # early-stream insertions: transposes + tail of the
                    # image projection (relu on DVE, PSUM from the AV pool)
                    if 1 <= G <= 2:
                        emit_transpose_group(u, G - 1)
                    elif G == 3:
                        # both image-tail groups as one parity-preserving
                        # pair of score-pool allocations
                        proj_group(u, qT[u], wimg_t, NCI_IMG, xi[u],
                                   CHUNK // 2, img_b, False)
                        proj_group(u, qT[u], wimg_t, NCI_IMG, xi[u],
                                   CHUNK // 2 + QB, img_b, False)
                else:
                    if G == NG and u + 1 < UPC:
                        emit_proj_part1(u + 1)
                J = G - GLA
                if 0 <= J < NG:
                    qbj, jslices = GRPS[J]
                    if J % 6 == 0:
                        pos[qbj] = av_psum.tile([128, QB], F32,
                                                name=f"po_{u}_{qbj}",
                                                tag="av")
                    for s in jslices:
                        kslice = kpw[u][s // 8][:, ts(s % 8, 128)]
                        mm_dedup(pos[qbj][:], kslice, ets[(qbj, s)][:],
                                 start=(s == 0), stop=(s == KSL - 1))
                    if jslices[-1] == KSL - 1:
                        qb_epilogue(qbj)

    nc.compile()
    return nc


def _shard_inputs(inputs):
    """Build the 8 per-core input maps from the full input dict."""
    mw = np.asarray(inputs["modality_weights"], np.float64)
    e = np.exp(mw - mw.max())
    w = (e / e.sum()).astype(np.float64)
    w0, w1 = float(w[0]), float(w[1])

    def bn_fold(gamma, beta, mean, var, mul):
        g = np.asarray(gamma, np.float64)
        b = np.asarray(beta, np.float64)
        m = np.asarray(mean, np.float64)
        v = np.asarray(var, np.float64)
        scale = g / np.sqrt(v + EPS) * mul
        bias = (b - m * g / np.sqrt(v + EPS)) * mul
        return scale, bias.astype(np.float32)

    i_s, i_b = bn_fold(inputs["image_gamma"], inputs["image_beta"],
                       inputs["image_mean"], inputs["image_var"], w0)
    l_s, l_b = bn_fold(inputs["lidar_gamma"], inputs["lidar_beta"],
                       inputs["lidar_mean"], inputs["lidar_var"], 1.0)

    # weight slices with the BN scale folded in, packed [cin(128), ci*CO]
    wi = (np.asarray(inputs["image_w"], np.float64) * i_s[:, None]).T.reshape(
        NCI_IMG, 128, CO).transpose(1, 0, 2).reshape(128, NCI_IMG * CO)
    wi = np.ascontiguousarray(wi).astype(NPBF16)
    wl = (np.asarray(inputs["lidar_w"], np.float64) * l_s[:, None]).T.reshape(
        NCI_LID, 128, CO).transpose(1, 0, 2).reshape(128, NCI_LID * CO)
    wl = np.ascontiguousarray(wl).astype(NPBF16)

    scal = np.zeros((128, 4), np.float32)
    scal[:, 0] = i_b
    scal[:, 1] = l_b
    scal[:, 2] = 1.0 / (w0 * math.sqrt(CO))
    scal[:, 3] = w1
    idon = np.zeros((128, 256), np.float32)
    idon[:, :128] = np.eye(128, dtype=np.float32)
    # ones carry 1/w1 so linv = 1/pl = w1/denominator
    idon[:, 128:] = 1.0 / w1
    idon = idon.astype(NPBF16)

    # full features reshaped to (B, nchunks, C, 2048), bf16
    img = np.asarray(inputs["image_features"], np.float32).reshape(
        B, CI, NCH, CHUNK).astype(NPBF16)
    lid = np.asarray(inputs["lidar_features"], np.float32).reshape(
        B, CL, NCH, CHUNK).astype(NPBF16)

    in_maps = []
    for core in range(NCORES):
        ximg = np.empty((UPC, NCI_IMG, 128, CHUNK), NPBF16)
        xlid = np.empty((UPC, NCI_LID, 128, CHUNK), NPBF16)
        for ul in range(UPC):
            un = core * UPC + ul
            b, c = un // NCH, un % NCH
            ximg[ul] = img[b, :, c, :].reshape(NCI_IMG, 128, CHUNK)
            xlid[ul] = lid[b, :, c, :].reshape(NCI_LID, 128, CHUNK)
        in_maps.append({
            "ximg": ximg, "xlid": xlid, "wimg": wi, "wlid": wl,
            "scal": scal, "idon": idon,
        })
    return in_maps


def kernel(**inputs) -> np.ndarray:
    global _PROGRAM, LAST_RESULTS
    if _PROGRAM is None:
        _PROGRAM = _build_program()
    nc = _PROGRAM

    in_maps = _shard_inputs(inputs)
    trace = os.environ.get("BASS_KERNEL_TRACE", "0") == "1"
    tmpdir = os.environ.get("BASS_KERNEL_TRACE_DIR") or None
    if tmpdir:
        os.makedirs(tmpdir, exist_ok=True)
    results = run_bass_kernel_spmd(nc, in_maps, core_ids=list(range(NCORES)),
                                   trace=trace, tmpdir=tmpdir)
    LAST_RESULTS = results

    out = np.empty((B, CO, H, W), np.float32)
    outv = out.reshape(B, CO, NCH, NQB, QB)
    for core in range(NCORES):
        yc = np.asarray(results.results[core]["y"], dtype=np.float32)
        for ul in range(UPC):
            un = core * UPC + ul
            b, c = un // NCH, un % NCH
            outv[b, :, c, :, :] = yc[ul].transpose(1, 0, 2)
    return out


if __name__ == "__main__":
    rng = np.random.default_rng(0)
    inputs = {
        "lidar_features": rng.standard_normal((B, CL, H, W), np.float32),
        "image_features": rng.standard_normal((B, CI, H, W), np.float32),
        "lidar_w": rng.standard_normal((CO, CL), np.float32) * np.sqrt(2.0 / CO),
        "lidar_gamma": np.ones(CO, np.float32),
        "lidar_beta": np.zeros(CO, np.float32),
        "lidar_mean": rng.standard_normal(CO).astype(np.float32) * 0.1,
        "lidar_var": rng.uniform(0.5, 1.5, CO).astype(np.float32),
        "image_w": rng.standard_normal((CO, CI), np.float32) * np.sqrt(2.0 / CO),
        "image_gamma": np.ones(CO, np.float32),
        "image_beta": np.zeros(CO, np.float32),
        "image_mean": rng.standard_normal(CO).astype(np.float32) * 0.1,
        "image_var": rng.uniform(0.5, 1.5, CO).astype(np.float32),
        "modality_weights": np.ones(2, np.float32),
    }
    out = kernel(**inputs)
    print("kernel out:", out.shape, out.dtype, float(np.abs(out).mean()))
